# revision 1
# baseline (speedup 1.0000x reference)
"""Weighted-BCE loss kernel for Trainium2 (8 NeuronCores, SPMD data-parallel).

Reference math (torch-style BCELoss with class-balancing weights):
    n   = len(x), s = sum(gt)
    w0  = n / (2*(n-s)),  w1 = n / (2*s)
    L1  = max(log(x),     -100)
    L0  = max(log1p(-x),  -100)
    loss = mean( where(gt==0, w0, w1) * -(gt*L1 + (1-gt)*L0) )

The weights depend only on the GLOBAL positive count s, so the loss
decomposes into 4 global sums computed shard-locally:
    A = sum(gt * L1),  B = sum(gt * L0),  C = sum(L0),  s = sum(gt)
    loss = -( A/(2s) + (C-B)/(2(n-s)) )

Each core processes a 1/8 shard laid out [128 partitions, 16384 free]:
  - ScalarE (ACT): Ln(x), and Ln(1-x) via the free affine (scale=-1,
    bias=1); the second op's accum_out produces C for free; a Copy
    activation of gt with accum_out produces s.  ACT also issues the gt
    DMAs so x and gt stream through two separate HWDGE queues.
  - VectorE (DVE): two fused scalar_tensor_tensor ops, each doing
    clamp(max, -100) + multiply-by-gt + row-reduce in one instruction
    (A and B).  gt (int32) is consumed directly as the in1 operand.
All engines stay near the DMA roofline (16.8 MB/core @ 358 GB/s ~ 47us).
Host gathers the [128, 4*ntiles] partials from all 8 cores and finishes
the (tiny) all-reduce + final scalar arithmetic in float64.
"""

import numpy as np
from contextlib import ExitStack

import concourse.bass as bass
import concourse.bacc as bacc
import concourse.mybir as mybir
import concourse.tile as tile
from concourse.alu_op_type import AluOpType
from concourse.bass_utils import run_bass_kernel_spmd

N_TOTAL = 16777216
N_CORES = 8
PER_CORE = N_TOTAL // N_CORES   # 2097152
P = 128
FD = PER_CORE // P              # 16384 free elements per partition
# uniform large tiles measured fastest: per-instruction + semaphore overhead
# of extra small tiles outweighs the ramp/tail savings they buy
TILE_SIZES = [4096, 4096, 4096, 4096]
assert sum(TILE_SIZES) == FD
NT = len(TILE_SIZES)
# s-sum runs on ACT (copy+accum) for every tile; all DVE-side s variants
# (including tile-0-only, where DVE idles during ramp) measured 4-10us slower
S_ON_ACT = {0, 1, 2, 3}
LOG_CLAMP = -100.0

# Optional instrumentation knobs for a driver script (harness never sets them).
TRACE = False
LAST_RESULTS = None

_NC_CACHE = None


def _build():
    f32 = mybir.dt.float32
    i32 = mybir.dt.int32
    Ln = mybir.ActivationFunctionType.Ln

    nc = bacc.Bacc("TRN2")
    x_in = nc.declare_dram_parameter("x", [P, FD], f32, isOutput=False)
    g_in = nc.declare_dram_parameter("gt", [P, FD], i32, isOutput=False)
    # one packed output: columns [A | B | C | S], NT each
    out_all = nc.declare_dram_parameter("out_all", [P, 4 * NT], f32, isOutput=True)

    with tile.TileContext(nc) as tc, ExitStack() as ctx:
        xp = ctx.enter_context(tc.tile_pool(name="xp", bufs=2))
        gp = ctx.enter_context(tc.tile_pool(name="gp", bufs=3))
        lp = ctx.enter_context(tc.tile_pool(name="lp", bufs=2))
        jp = ctx.enter_context(tc.tile_pool(name="jp", bufs=1))
        accp = ctx.enter_context(tc.tile_pool(name="accp", bufs=1))

        accA = accp.tile([P, NT], f32)
        accB = accp.tile([P, NT], f32)
        accC = accp.tile([P, NT], f32)
        accS = accp.tile([P, NT], f32)
        groups = [accA, accB, accC, accS]

        def col(group, i):
            return groups[group][:, i : i + 1]

        off = 0
        for i, tfd in enumerate(TILE_SIZES):
            sl = slice(off, off + tfd)
            off += tfd
            xt = xp.tile([P, tfd], f32, tag="xt")
            gt_t = gp.tile([P, tfd], i32, tag="gt")
            # two HWDGE queues: x via SP(sync), gt via the ACT sequencer
            nc.sync.dma_start(xt[:], x_in[:, sl])
            nc.scalar.dma_start(gt_t[:], g_in[:, sl])

            lnx = lp.tile([P, tfd], f32, tag="lnx")
            ln1 = lp.tile([P, tfd], f32, tag="ln1")
            nc.scalar.activation(lnx[:], xt[:], Ln)
            nc.scalar.activation(
                ln1[:], xt[:], Ln, bias=1.0, scale=-1.0,
                accum_out=col(2, i),
            )

            junk = jp.tile([P, tfd], f32, tag="junk")
            nc.vector.scalar_tensor_tensor(
                junk[:], lnx[:], LOG_CLAMP, gt_t[:],
                AluOpType.max, AluOpType.mult,
                accum_out=col(0, i),
            )
            junk2 = jp.tile([P, tfd], f32, tag="junk")
            nc.vector.scalar_tensor_tensor(
                junk2[:], ln1[:], LOG_CLAMP, gt_t[:],
                AluOpType.max, AluOpType.mult,
                accum_out=col(1, i),
            )
            # s = sum(gt), load-balanced between ACT (copy+accum) and DVE
            # (STT: (junk*0) + gt with accum; junk is finite by construction)
            junk3 = jp.tile([P, tfd], f32, tag="junk3")
            if i in S_ON_ACT:
                nc.scalar.activation(
                    junk3[:], gt_t[:], mybir.ActivationFunctionType.Copy,
                    accum_out=col(3, i),
                )
            else:
                nc.vector.scalar_tensor_tensor(
                    junk3[:], junk[:], 0.0, gt_t[:],
                    AluOpType.mult, AluOpType.add,
                    accum_out=col(3, i),
                )

        for k, g in enumerate(groups):
            nc.sync.dma_start(out_all[:, k * NT : (k + 1) * NT], g[:])

    nc.compile()
    return nc


def get_nc():
    global _NC_CACHE
    if _NC_CACHE is None:
        _NC_CACHE = _build()
    return _NC_CACHE


def make_in_maps(x, gt):
    x = np.ascontiguousarray(np.asarray(x, dtype=np.float32).reshape(-1))
    gt = np.ascontiguousarray(np.asarray(gt, dtype=np.int32).reshape(-1))
    assert x.shape == (N_TOTAL,) and gt.shape == (N_TOTAL,)
    in_maps = []
    for c in range(N_CORES):
        sl = slice(c * PER_CORE, (c + 1) * PER_CORE)
        in_maps.append({
            "x": x[sl].reshape(P, FD),
            "gt": gt[sl].reshape(P, FD),
        })
    return in_maps


def combine(results):
    """All-reduce the per-core partial sums and finish the loss formula."""
    A = B = C = S = 0.0
    for r in results:
        o = r["out_all"].astype(np.float64)
        A += o[:, 0 * NT : 1 * NT].sum()
        B += o[:, 1 * NT : 2 * NT].sum()
        C += o[:, 2 * NT : 3 * NT].sum()
        S += o[:, 3 * NT : 4 * NT].sum()
    n = float(N_TOTAL)
    result = -(A / (2.0 * S) + (C - B) / (2.0 * (n - S)))
    return np.array(result, dtype=np.float32)


def kernel(x, gt):
    global LAST_RESULTS
    nc = get_nc()
    in_maps = make_in_maps(x, gt)
    br = run_bass_kernel_spmd(nc, in_maps, list(range(N_CORES)))
    LAST_RESULTS = br
    return combine(br.results)



# revision 3
# speedup vs baseline: 1.1898x; 1.1898x over previous
"""Weighted-BCE loss kernel for Trainium2 (8 NeuronCores, SPMD data-parallel).

Reference math (torch-style BCELoss with class-balancing weights):
    n   = len(x), s = sum(gt)
    w0  = n / (2*(n-s)),  w1 = n / (2*s)
    L1  = max(log(x),     -100)
    L0  = max(log1p(-x),  -100)
    loss = mean( where(gt==0, w0, w1) * -(gt*L1 + (1-gt)*L0) )

The weights depend only on the GLOBAL positive count s, so the loss
decomposes into 3 global sums computed shard-locally:
    A  = sum(gt * L1)                (only positives contribute)
    D' = sum((gt-1) * L0u) = B - C   (only negatives contribute, negated)
    s  = sum(gt)
    loss = -A/(2s) + D'/(2(n-s))
L0u is UNclamped log(1-x): x is fp32 in [0,1), so 1-x >= 2^-25 and
log1p(-x) >= -17.4 — the -100 clamp can never fire on the L0 branch.
The L1 clamp is kept (x == 0 occurs with prob ~1 in 2^24 per element).

Engine split per 1/8 shard (2M elements as [128 partitions, 16384 free]):
  - gt is narrowed to bf16 on the host (0/1 — exact): cuts its DMA
    traffic in half and unlocks the DVE's 2x/4x bf16 perf modes.
  - ScalarE (ACT): exactly two Ln passes per tile — ln(1-x) via the free
    affine (scale=-1, bias=1) and ln(x) — writing bf16.  No accum_out on
    ACT: no accumulator drains/reads on what used to be the pacer engine.
  - VectorE (DVE): three reduce ops per tile, all on bf16 inputs:
      s  via tensor_scalar  (gt mult 1)            + accum   (4x mode)
      D' via STT            (gt sub 1) mult ln1    + accum   (2x mode)
      A  via STT            (lnx max -100) mult gt + accum   (2x mode)
  - All input DMA on the sync HWDGE ring (gt tile then x tile per wave);
    compute engines issue no DMAs.  Deep buffering (bufs=4) keeps DMA
    descriptors always eligible — the kernel runs at the HBM roofline
    (12.6 MB/core @ ~358 GB/s ~ 35 us) instead of being compute-gated.
  - First/last tiles are small to shrink pipeline ramp and drain.
Host gathers the [128, 3*NT] partials from all 8 cores and finishes the
(tiny) all-reduce + final scalar arithmetic in float64.
"""

import numpy as np
import ml_dtypes
from contextlib import ExitStack

import concourse.bass as bass
import concourse.bacc as bacc
import concourse.mybir as mybir
import concourse.tile as tile
from concourse.alu_op_type import AluOpType
from concourse.bass_utils import run_bass_kernel_spmd

N_TOTAL = 16777216
N_CORES = 8
PER_CORE = N_TOTAL // N_CORES   # 2097152
P = 128
FD = PER_CORE // P              # 16384 free elements per partition
TILE_SIZES = [1024, 5120, 5120, 4096, 1024]
assert sum(TILE_SIZES) == FD
NT = len(TILE_SIZES)
LOG_CLAMP = -100.0

# Optional instrumentation knobs for a driver script (harness never sets them).
TRACE = False
LAST_RESULTS = None

_NC_CACHE = None


def _build():
    f32 = mybir.dt.float32
    bf16 = mybir.dt.bfloat16
    Ln = mybir.ActivationFunctionType.Ln

    nc = bacc.Bacc("TRN2")
    x_in = nc.declare_dram_parameter("x", [P, FD], f32, isOutput=False)
    g_in = nc.declare_dram_parameter("gt", [P, FD], bf16, isOutput=False)
    # one packed output: columns [A | D | S], NT each
    out_all = nc.declare_dram_parameter("out_all", [P, 3 * NT], f32, isOutput=True)

    with tile.TileContext(nc) as tc, ExitStack() as ctx:
        xp = ctx.enter_context(tc.tile_pool(name="xp", bufs=4))
        gp = ctx.enter_context(tc.tile_pool(name="gp", bufs=4))
        lp = ctx.enter_context(tc.tile_pool(name="lp", bufs=2))
        jp = ctx.enter_context(tc.tile_pool(name="jp", bufs=1))
        accp = ctx.enter_context(tc.tile_pool(name="accp", bufs=1))

        accA = accp.tile([P, NT], f32)
        accD = accp.tile([P, NT], f32)
        accS = accp.tile([P, NT], f32)
        groups = [accA, accD, accS]

        def col(group, i):
            return groups[group][:, i : i + 1]

        off = 0
        for i, tfd in enumerate(TILE_SIZES):
            sl = slice(off, off + tfd)
            off += tfd
            xt = xp.tile([P, tfd], f32, tag="xt")
            gt_t = gp.tile([P, tfd], bf16, tag="gt")
            # single HWDGE ring (sync): gt first so DVE's s-op can start early
            nc.sync.dma_start(gt_t[:], g_in[:, sl])
            nc.sync.dma_start(xt[:], x_in[:, sl])

            # s = sum(gt): single-src tensor_scalar, 4x bf16 mode
            junk_s = jp.tile([P, tfd], bf16, tag="junk")
            nc.vector.tensor_scalar(
                junk_s[:], gt_t[:], 1.0, 0.0, AluOpType.mult, AluOpType.add,
                accum_out=col(2, i),
            )

            ln1 = lp.tile([P, tfd], bf16, tag="ln1")
            nc.scalar.activation(ln1[:], xt[:], Ln, bias=1.0, scale=-1.0)
            # D' = sum((gt-1) * ln(1-x)) = B - C  (negatives-only mass, negated)
            junk_d = jp.tile([P, tfd], bf16, tag="junk")
            nc.vector.scalar_tensor_tensor(
                junk_d[:], gt_t[:], 1.0, ln1[:],
                AluOpType.subtract, AluOpType.mult,
                accum_out=col(1, i),
            )

            lnx = lp.tile([P, tfd], bf16, tag="lnx")
            nc.scalar.activation(lnx[:], xt[:], Ln)
            # A = sum(gt * max(ln x, -100))
            junk_a = jp.tile([P, tfd], bf16, tag="junk")
            nc.vector.scalar_tensor_tensor(
                junk_a[:], lnx[:], LOG_CLAMP, gt_t[:],
                AluOpType.max, AluOpType.mult,
                accum_out=col(0, i),
            )

        for k, g in enumerate(groups):
            nc.sync.dma_start(out_all[:, k * NT : (k + 1) * NT], g[:])

    nc.compile()
    return nc


def get_nc():
    global _NC_CACHE
    if _NC_CACHE is None:
        _NC_CACHE = _build()
    return _NC_CACHE


def make_in_maps(x, gt):
    x = np.ascontiguousarray(np.asarray(x, dtype=np.float32).reshape(-1))
    gt = np.asarray(gt).reshape(-1)
    assert x.shape == (N_TOTAL,) and gt.shape == (N_TOTAL,)
    # narrow the 0/1 labels to bf16 (exact) for half the DMA traffic and
    # the DVE's packed 16-bit perf modes
    gtb = np.ascontiguousarray(gt.astype(ml_dtypes.bfloat16))
    in_maps = []
    for c in range(N_CORES):
        sl = slice(c * PER_CORE, (c + 1) * PER_CORE)
        in_maps.append({
            "x": x[sl].reshape(P, FD),
            "gt": gtb[sl].reshape(P, FD),
        })
    return in_maps


def combine(results):
    """All-reduce the per-core partial sums and finish the loss formula."""
    A = D = S = 0.0
    for r in results:
        o = r["out_all"].astype(np.float64)
        A += o[:, 0 * NT : 1 * NT].sum()
        D += o[:, 1 * NT : 2 * NT].sum()
        S += o[:, 2 * NT : 3 * NT].sum()
    n = float(N_TOTAL)
    result = -A / (2.0 * S) + D / (2.0 * (n - S))
    return np.array(result, dtype=np.float32)


def kernel(x, gt):
    global LAST_RESULTS
    nc = get_nc()
    in_maps = make_in_maps(x, gt)
    br = run_bass_kernel_spmd(nc, in_maps, list(range(N_CORES)))
    LAST_RESULTS = br
    return combine(br.results)
